# revision 1
# baseline (speedup 1.0000x reference)
"""GAT (2-layer, PyG-style) forward on 8 TRN2 NeuronCores.

Sharding: dst-node blocks across cores; per-core edge lists routed/sorted by
dst block on host; per-edge payload gathered from a replicated node table via
indirect DMA; segment softmax + weighted sum via selection-matrix matmuls in
PSUM."""
import sys
if '/opt/trn_rl_repo' not in sys.path:
    sys.path.insert(0, '/opt/trn_rl_repo')
import json
import numpy as np
import ml_dtypes

import concourse.bass as bass
import concourse.mybir as mybir
import concourse.tile as tile

bf16 = ml_dtypes.bfloat16
F32 = mybir.dt.float32
BF16 = mybir.dt.bfloat16
I32 = mybir.dt.int32
ALU = mybir.AluOpType
ACTF = mybir.ActivationFunctionType


def mkap(ap, dims, elem_offset=0):
    """AP with explicit [step, count] free dims (elements) after the partition dim."""
    return bass.AP(ap.tensor, ap.offset + elem_offset,
                   [list(ap.ap[0])] + [list(d) for d in dims])


def build_gat_layer(N, FIN, H, C, T_blk, n_blocks, core_rows, mode, debug=False, ablate=None):
    """mode: 'elu' (layer 1) or 'mean_lsm' (layer 2)."""
    FOUT = H * C
    TCOLS = FOUT + 2 * H          # table row: [h | al_src | al_dst]
    MCOLS = FOUT + H              # m' row: [ee*h | ee]
    NT = n_blocks * T_blk
    KCH = FIN // 128
    n_tiles = (N + 127) // 128
    last_blk_rows = core_rows - (n_blocks - 1) * 128

    nc = bass.Bass("TRN2", target_bir_lowering=False, debug=False, num_devices=8)

    xT = nc.dram_tensor("xT", [128, KCH, N], BF16, kind="ExternalInput")
    wcat = nc.dram_tensor("wcat", [128, KCH, TCOLS], BF16, kind="ExternalInput")
    bias_cols = FOUT if mode == "elu" else C
    bias_in = nc.dram_tensor("bias", [128, bias_cols], F32, kind="ExternalInput")
    iota_in = nc.dram_tensor("iota", [128, 128], BF16, kind="ExternalInput")
    srcidx_in = nc.dram_tensor("srcidx", [128, NT], I32, kind="ExternalInput")
    dstloc_in = nc.dram_tensor("dstloc", [128, NT], BF16, kind="ExternalInput")
    dstlocT_in = nc.dram_tensor("dstlocT", [NT, 128], BF16, kind="ExternalInput")
    iotac_in = nc.dram_tensor("iotac", [128, 1], BF16, kind="ExternalInput")
    adrow_in = nc.dram_tensor("adrow", [128, n_blocks], I32, kind="ExternalInput")
    if mode == "elu":
        out_d = nc.dram_tensor("out", [core_rows, FOUT], BF16, kind="ExternalOutput")
    else:
        out_d = nc.dram_tensor("out", [core_rows, C], F32, kind="ExternalOutput")
    table = nc.dram_tensor("table", [N, TCOLS], BF16)
    if False:
        dbg_g = nc.dram_tensor("dbg_g", [128, T_blk * TCOLS], BF16, kind="ExternalOutput")
        dbg_s01 = nc.dram_tensor("dbg_s01", [128, T_blk * 128], BF16, kind="ExternalOutput")
        dbg_ee = nc.dram_tensor("dbg_ee", [128, T_blk * H], BF16, kind="ExternalOutput")
        dbg_mp = nc.dram_tensor("dbg_mp", [128, T_blk * MCOLS], BF16, kind="ExternalOutput")
        dbg_up = nc.dram_tensor("dbg_up", [128, MCOLS], F32, kind="ExternalOutput")

    ST = 8                       # node tiles per staging buffer / table-write DMA
    CH = ST * 128                # xT chunk columns
    n_ch = (N + CH - 1) // CH

    with tile.TileContext(nc) as tc:
        with (
            tc.tile_pool(name="const", bufs=1) as kpool,
            tc.tile_pool(name="xchunk", bufs=3) as xpool,
            tc.tile_pool(name="stage", bufs=3) as stpool,
            tc.tile_pool(name="dpsum", bufs=2, space="PSUM") as dppool,
            tc.tile_pool(name="g", bufs=2) as gpool,
            tc.tile_pool(name="s01", bufs=2) as spool,
            tc.tile_pool(name="ee", bufs=2) as eepool,
            tc.tile_pool(name="mp", bufs=2) as mppool,
            tc.tile_pool(name="upsum", bufs=2, space="PSUM") as uppool,
            tc.tile_pool(name="epi", bufs=2) as epool,
        ):
            # ---- constants ----
            wcat_sb = kpool.tile([128, KCH * TCOLS], BF16)
            nc.sync.dma_start(out=wcat_sb[:], in_=wcat[:].rearrange("p k c -> p (k c)"))
            bias_sb = kpool.tile([128, bias_cols], F32)
            nc.sync.dma_start(out=bias_sb[:], in_=bias_in[:])
            iota_sb = kpool.tile([128, 128], BF16)
            nc.sync.dma_start(out=iota_sb[:], in_=iota_in[:])
            srcidx_sb = kpool.tile([128, NT], I32)
            nc.sync.dma_start(out=srcidx_sb[:], in_=srcidx_in[:])
            dstloc_sb = kpool.tile([128, NT], BF16)
            nc.sync.dma_start(out=dstloc_sb[:], in_=dstloc_in[:])
            iotac_sb = kpool.tile([128, 1], BF16)
            nc.sync.dma_start(out=iotac_sb[:], in_=iotac_in[:])
            adrow_sb = kpool.tile([128, n_blocks], I32)
            nc.sync.dma_start(out=adrow_sb[:], in_=adrow_in[:])

            # ---- dense phase: table[N, TCOLS] = [x @ Wcat] ----
            for ci in range(n_ch):
                c0 = ci * CH
                ccols = min(CH, N - c0)
                nt_ch = (ccols + 127) // 128
                xc = xpool.tile([128, KCH * CH], BF16, tag="xc")
                nc.sync.dma_start(
                    out=mkap(xc[:], [[CH, KCH], [1, ccols]]),
                    in_=xT[:, :, c0:c0 + ccols],
                )
                st = stpool.tile([128, ST * TCOLS], BF16, tag="st")
                for tl in range(nt_ch):
                    rows = min(128, ccols - tl * 128)
                    col = tl * 128
                    psum = dppool.tile([128, TCOLS], F32, tag="dp")
                    for k in range(KCH):
                        nc.tensor.matmul(
                            psum[:rows, :],
                            lhsT=xc[:, k * CH + col: k * CH + col + rows],
                            rhs=wcat_sb[:, k * TCOLS:(k + 1) * TCOLS],
                            start=(k == 0),
                            stop=(k == KCH - 1),
                        )
                    nc.vector.tensor_copy(
                        out=st[:rows, tl * TCOLS:(tl + 1) * TCOLS], in_=psum[:rows, :])
                # flush: full 128-row tiles in one strided DMA, partial tail separately
                n_full = ccols // 128
                if n_full:
                    nc.sync.dma_start(
                        out=table[c0:c0 + n_full * 128, :]
                        .rearrange("(a p) c -> p a c", p=128),
                        in_=mkap(st[:], [[TCOLS, n_full], [1, TCOLS]]),
                    )
                rem = ccols - n_full * 128
                if rem:
                    nc.sync.dma_start(
                        out=table[c0 + n_full * 128: c0 + ccols, :],
                        in_=st[:rem, n_full * TCOLS:(n_full + 1) * TCOLS],
                    )

            # ---- edge phase ----
            for b in range(n_blocks):
                brows = 128 if b < n_blocks - 1 else last_blk_rows
                GCOLS = FOUT + H
                g_blk = gpool.tile([128, T_blk * GCOLS], BF16, tag="g")
                for t in range(T_blk):
                    j = b * T_blk + t
                    if ablate != "gather":
                        nc.gpsimd.indirect_dma_start(
                            out=g_blk[:, t * GCOLS:(t + 1) * GCOLS],
                            out_offset=None,
                            in_=table[:],
                            in_offset=bass.IndirectOffsetOnAxis(ap=srcidx_sb[:, j:j + 1], axis=0),
                        )
                # s01[j, t*128 + d] = (dstloc[j, b*T+t] == d)
                s01 = spool.tile([128, T_blk * 128], BF16, tag="s01")
                nc.vector.tensor_tensor(
                    out=s01[:],
                    in0=mkap(dstloc_sb[:], [[1, T_blk], [0, 128]], elem_offset=b * T_blk),
                    in1=mkap(iota_sb[:], [[0, T_blk], [1, 128]]),
                    op=ALU.is_equal,
                )
                # S01T[d, (t,j)] = (dstloc[j,t] == d) via partition-bcast load of dstlocT
                rep = spool.tile([128, T_blk * 128], BF16, tag="rep")
                nc.sync.dma_start(
                    out=rep[:],
                    in_=bass.AP(dstlocT_in[:].tensor, b * T_blk * 128,
                                [[0, 128], [1, T_blk * 128]]),
                )
                s01T = spool.tile([128, T_blk * 128], BF16, tag="s01T")
                nc.vector.tensor_tensor(
                    out=s01T[:],
                    in0=rep[:],
                    in1=mkap(iotac_sb[:], [[0, T_blk], [0, 128]]),
                    op=ALU.is_equal,
                )
                # ad_blk: gather own dst rows of the table (cols FOUT+H..) and
                # broadcast to edges: ad_pe[j, (t,h)] = sum_d S01T[d,(t,j)] * ad_blk[d,h]
                adrow = gpool.tile([128, TCOLS], BF16, tag="adrow")
                nc.gpsimd.indirect_dma_start(
                    out=adrow[:], out_offset=None, in_=table[:],
                    in_offset=bass.IndirectOffsetOnAxis(ap=adrow_sb[:, b:b + 1], axis=0),
                )
                adp = uppool.tile([128, T_blk * H], F32, tag="adp")
                for t in range(T_blk):
                    nc.tensor.matmul(
                        adp[:, t * H:(t + 1) * H],
                        lhsT=s01T[:, t * 128:(t + 1) * 128],
                        rhs=adrow[:, FOUT + H:FOUT + 2 * H],
                        start=True, stop=True,
                    )
                # s = as + ad_pe ; lrelu ; ee = exp
                s_f = eepool.tile([128, T_blk * H], F32, tag="sf")
                nc.vector.tensor_tensor(
                    out=s_f[:],
                    in0=mkap(g_blk[:], [[GCOLS, T_blk], [1, H]], elem_offset=FOUT),
                    in1=adp[:],
                    op=ALU.add,
                )
                s2 = eepool.tile([128, T_blk * H], F32, tag="s2")
                nc.vector.tensor_scalar_mul(out=s2[:], in0=s_f[:], scalar1=0.2)
                nc.vector.tensor_tensor(out=s_f[:], in0=s_f[:], in1=s2[:], op=ALU.max)
                ee = eepool.tile([128, T_blk * H], BF16, tag="ee")
                nc.scalar.activation(ee[:], s_f[:], ACTF.Exp)
                # m' = [ee*h | ee]
                mp = mppool.tile([128, T_blk * MCOLS], BF16, tag="mp")
                nc.vector.tensor_tensor(
                    out=mkap(mp[:], [[MCOLS, T_blk], [1, FOUT]]),
                    in0=mkap(g_blk[:], [[GCOLS, T_blk], [1, FOUT]]),
                    in1=mkap(ee[:], [[H, T_blk], [1, H], [0, C]]),
                    op=ALU.mult,
                )
                nc.vector.tensor_copy(
                    out=mkap(mp[:], [[MCOLS, T_blk], [1, H]], elem_offset=FOUT),
                    in_=ee[:],
                )
                # segment-sum matmuls into PSUM
                up = uppool.tile([128, MCOLS], F32, tag="up")
                for t in range(T_blk):
                    if ablate == "mm":
                        break
                    nc.tensor.matmul(
                        up[:],
                        lhsT=s01[:, t * 128:(t + 1) * 128],
                        rhs=mp[:, t * MCOLS:(t + 1) * MCOLS],
                        start=(t == 0),
                        stop=(t == T_blk - 1),
                    )
                if False:
                    nc.sync.dma_start(out=dbg_g[:], in_=g_blk[:])
                    nc.sync.dma_start(out=dbg_s01[:], in_=s01[:])
                    nc.sync.dma_start(out=dbg_ee[:], in_=ee[:])
                    nc.sync.dma_start(out=dbg_mp[:], in_=mp[:])
                    up_c = epool.tile([128, MCOLS], F32, tag="upc")
                    nc.vector.tensor_copy(out=up_c[:], in_=up[:])
                    nc.sync.dma_start(out=dbg_up[:], in_=up_c[:])
                # ---- epilogue ----
                rec = epool.tile([128, H], F32, tag="rec")
                nc.vector.reciprocal(out=rec[:brows], in_=up[:brows, FOUT:FOUT + H])
                u = epool.tile([128, FOUT], F32, tag="u")
                nc.vector.tensor_tensor(
                    out=u[:brows], in0=up[:brows, 0:FOUT],
                    in1=mkap(rec[:brows], [[1, H], [0, C]]),
                    op=ALU.mult,
                )
                if mode == "elu":
                    hb = epool.tile([128, FOUT], F32, tag="hb")
                    nc.vector.tensor_tensor(out=hb[:brows], in0=u[:brows],
                                            in1=bias_sb[:brows], op=ALU.add)
                    mn = epool.tile([128, FOUT], F32, tag="mn")
                    nc.vector.tensor_scalar_min(out=mn[:brows], in0=hb[:brows], scalar1=0.0)
                    ex = epool.tile([128, FOUT], F32, tag="ex")
                    nc.scalar.activation(ex[:brows], mn[:brows], ACTF.Exp)
                    mx = epool.tile([128, FOUT], F32, tag="mx")
                    nc.vector.tensor_scalar_max(out=mx[:brows], in0=hb[:brows], scalar1=0.0)
                    sm = epool.tile([128, FOUT], F32, tag="sm")
                    nc.vector.tensor_tensor(out=sm[:brows], in0=mx[:brows], in1=ex[:brows],
                                            op=ALU.add)
                    ob = epool.tile([128, FOUT], BF16, tag="ob")
                    nc.vector.tensor_scalar_sub(out=ob[:brows], in0=sm[:brows], scalar1=1.0)
                    nc.sync.dma_start(out=out_d[b * 128: b * 128 + brows, :], in_=ob[:brows])
                else:
                    m1 = epool.tile([128, FOUT // 2], F32, tag="m1")
                    nc.vector.tensor_tensor(out=m1[:brows], in0=u[:brows, 0:FOUT // 2],
                                            in1=u[:brows, FOUT // 2:FOUT], op=ALU.add)
                    m2 = epool.tile([128, FOUT // 4], F32, tag="m2")
                    nc.vector.tensor_tensor(out=m2[:brows], in0=m1[:brows, 0:FOUT // 4],
                                            in1=m1[:brows, FOUT // 4:FOUT // 2], op=ALU.add)
                    m3 = epool.tile([128, C], F32, tag="m3")
                    nc.vector.tensor_tensor(out=m3[:brows], in0=m2[:brows, 0:C],
                                            in1=m2[:brows, C:2 * C], op=ALU.add)
                    zb = epool.tile([128, C], F32, tag="zb")
                    nc.vector.tensor_scalar_mul(out=zb[:brows], in0=m3[:brows], scalar1=1.0 / H)
                    nc.vector.tensor_tensor(out=zb[:brows], in0=zb[:brows], in1=bias_sb[:brows],
                                            op=ALU.add)
                    mxr = epool.tile([128, 1], F32, tag="mxr")
                    nc.vector.reduce_max(out=mxr[:brows], in_=zb[:brows], axis=mybir.AxisListType.X)
                    xm = epool.tile([128, C], F32, tag="xm")
                    nc.vector.tensor_tensor(out=xm[:brows], in0=zb[:brows],
                                            in1=mkap(mxr[:brows], [[0, C]]),
                                            op=ALU.subtract)
                    exs = epool.tile([128, C], F32, tag="exs")
                    nc.scalar.activation(exs[:brows], xm[:brows], ACTF.Exp)
                    sms = epool.tile([128, 1], F32, tag="sms")
                    nc.vector.reduce_sum(out=sms[:brows], in_=exs[:brows], axis=mybir.AxisListType.X)
                    lg = epool.tile([128, 1], F32, tag="lg")
                    nc.scalar.activation(lg[:brows], sms[:brows], ACTF.Ln)
                    outt = epool.tile([128, C], F32, tag="outt")
                    nc.vector.tensor_tensor(out=outt[:brows], in0=xm[:brows],
                                            in1=mkap(lg[:brows], [[0, C]]),
                                            op=ALU.subtract)
                    nc.sync.dma_start(out=out_d[b * 128: b * 128 + brows, :], in_=outt[:brows])
    return nc


# ---------------- host side ----------------

def fold_weights(W, a_src, a_dst, H, C):
    """Wcat [FIN, H*C + 2H] f32: [W.T | Wa_src | Wa_dst]."""
    WT = np.asarray(W, np.float32).T.copy()           # [FIN, H*C]
    FIN = WT.shape[0]
    W3 = WT.reshape(FIN, H, C)
    Wa_s = np.einsum('fhc,hc->fh', W3, np.asarray(a_src, np.float32))
    Wa_d = np.einsum('fhc,hc->fh', W3, np.asarray(a_dst, np.float32))
    return np.concatenate([WT, Wa_s, Wa_d], axis=1)


def pack_kdim(M):
    """[FIN, COLS] -> [128, KCH, COLS]: row k*128+p -> [p, k]."""
    FIN, COLS = M.shape
    KCH = FIN // 128
    return np.ascontiguousarray(M.reshape(KCH, 128, COLS).transpose(1, 0, 2))


def route_edges(src, dst, n_cores, core_rows, n_nodes):
    """Returns (T_blk, per-core list of (srcidx [128,NT] i32, dstloc [128,NT] bf16))."""
    n_blocks = (core_rows + 127) // 128
    core_of = dst // core_rows
    counts = []
    per_core = []
    for c in range(n_cores):
        m = core_of == c
        s_c = src[m]
        dl = dst[m] - c * core_rows
        blk = np.minimum(dl // 128, n_blocks - 1)
        loc = dl - blk * 128
        order = np.lexsort((s_c, blk))
        s_c, blk, loc = s_c[order], blk[order], loc[order]
        per_core.append((s_c, blk, loc))
        counts.append(np.bincount(blk, minlength=n_blocks))
    T_blk = max(1, int(max(int(np.ceil(cnt.max() / 128.0)) for cnt in counts)))
    out = []
    for c in range(n_cores):
        s_c, blk, loc = per_core[c]
        cap = T_blk * 128
        sidx = np.zeros((n_blocks, cap), np.int32)
        dloc = np.full((n_blocks, cap), -1.0, np.float32)
        for b in range(n_blocks):
            m = blk == b
            k = int(m.sum())
            assert k <= cap
            sidx[b, :k] = s_c[m]
            dloc[b, :k] = loc[m]
        # [n_blocks, T_blk, 128] -> [128, n_blocks*T_blk]
        dlocT = dloc.reshape(n_blocks * T_blk, 128).astype(bf16)
        sidx = sidx.reshape(n_blocks * T_blk, 128).T.copy()
        dloc = dloc.reshape(n_blocks * T_blk, 128).T.copy()
        base = c * core_rows
        adrow = np.zeros((128, n_blocks), np.int32)
        for b in range(n_blocks):
            brows = min(128, core_rows - b * 128)
            pp = np.minimum(np.arange(128), brows - 1)
            adrow[:, b] = base + b * 128 + pp
        out.append({"srcidx": np.ascontiguousarray(sidx),
                    "dstloc": np.ascontiguousarray(dloc.astype(bf16)),
                    "dstlocT": np.ascontiguousarray(dlocT),
                    "adrow": adrow})
    return T_blk, n_blocks, out



MAX_WAITS = 1

def fix_excess_waits(nc):
    """Post-process BIR JSON: any instruction with >MAX_WAITS sem-waits gets
    preceding Nop instructions carrying the excess waits (same engine, in-order).
    Monkeypatches nc.to_json_bytes to return the fixed JSON."""
    raw = nc.to_json_bytes()
    d = json.loads(raw)
    n_fix = 0
    for f in d["functions"]:
        for bb in f["blocks"]:
            out = []
            for inst in bb["instructions"]:
                si = inst.get("sync_info")
                waits = (si or {}).get("on_wait") or []
                if len(waits) > MAX_WAITS:
                    extra = waits[:-MAX_WAITS]
                    keep = waits[-MAX_WAITS:]
                    for ci in range(0, len(extra), MAX_WAITS):
                        chunk = extra[ci:ci+MAX_WAITS]
                        n_fix += 1
                        out.append({
                            "debug": inst.get("debug", 0),
                            "engine": inst["engine"],
                            "ins": [],
                            "is_reset_sema": False,
                            "name": f"{inst['name']}-wfix{ci}",
                            "opcode": "EventSemaphore",
                            "outs": [],
                            "sync_info": {"on_update": [], "on_wait": chunk},
                        })
                    si["on_wait"] = keep
                out.append(inst)
            bb["instructions"] = out
    fixed = json.dumps(d).encode()
    nc.to_json_bytes = lambda: fixed
    return n_fix


# ---------------- top-level kernel ----------------

N_NODES = 50000
N_CORES = 8
CORE_ROWS = N_NODES // N_CORES
_CACHE = {}


def _get_program(key, builder):
    if key not in _CACHE:
        nc = builder()
        fix_excess_waits(nc)
        _CACHE[key] = nc
    return _CACHE[key]


def kernel(x, edge_index, W1, a_src1, a_dst1, b1, W2, a_src2, a_dst2, b2):
    from concourse.bass_utils import run_bass_kernel_spmd

    x = np.asarray(x, np.float32)
    ei = np.asarray(edge_index)
    N = N_NODES
    src = np.concatenate([ei[0], np.arange(N)]).astype(np.int64)
    dst = np.concatenate([ei[1], np.arange(N)]).astype(np.int64)
    T_blk, n_blocks, routed = route_edges(src, dst, N_CORES, CORE_ROWS, N)

    iota_rows = np.tile(np.arange(128, dtype=np.float32)[None, :], (128, 1)).astype(bf16)
    iota_col = np.arange(128, dtype=np.float32)[:, None].astype(bf16)

    # ---- layer 1 ----
    H1, C1 = 8, 32
    Wcat1 = fold_weights(W1, a_src1, a_dst1, H1, C1)
    nc1 = _get_program(("l1", T_blk), lambda: build_gat_layer(
        N, 128, H1, C1, T_blk, n_blocks, CORE_ROWS, "elu"))
    com1 = {
        "xT": pack_kdim(np.ascontiguousarray(x.T)).astype(bf16),
        "wcat": pack_kdim(Wcat1).astype(bf16),
        "bias": np.tile(np.asarray(b1, np.float32)[None, :], (128, 1)),
        "iota": iota_rows, "iotac": iota_col,
    }
    in_maps1 = [dict(com1, **routed[c]) for c in range(N_CORES)]
    res1 = run_bass_kernel_spmd(nc1, in_maps1, list(range(N_CORES)))
    h1 = np.concatenate([np.asarray(res1.results[c]["out"]) for c in range(N_CORES)], axis=0)

    # ---- layer 2 ----
    H2, C2 = 8, 40
    Wcat2 = fold_weights(W2, a_src2, a_dst2, H2, C2)
    nc2 = _get_program(("l2", T_blk), lambda: build_gat_layer(
        N, 256, H2, C2, T_blk, n_blocks, CORE_ROWS, "mean_lsm"))
    com2 = {
        "xT": pack_kdim(np.ascontiguousarray(np.float32(h1).T)).astype(bf16),
        "wcat": pack_kdim(Wcat2).astype(bf16),
        "bias": np.tile(np.asarray(b2, np.float32)[None, :], (128, 1)),
        "iota": iota_rows, "iotac": iota_col,
    }
    in_maps2 = [dict(com2, **routed[c]) for c in range(N_CORES)]
    res2 = run_bass_kernel_spmd(nc2, in_maps2, list(range(N_CORES)))
    out = np.concatenate([np.asarray(res2.results[c]["out"]) for c in range(N_CORES)], axis=0)
    return out.astype(np.float32)



# revision 43
# speedup vs baseline: 1.1739x; 1.1739x over previous
"""GAT (2-layer, PyG-style) forward on 8 TRN2 NeuronCores.

Sharding: dst-node blocks across cores (host-permuted for per-block edge-count
balance); per-core edge lists routed by dst block on host; per-edge payload
gathered from a replicated node table via one batched indirect DMA per block;
al_dst broadcast via a second tiny indirect gather on the dst indices; segment
softmax + weighted sum via selection-matrix matmuls in PSUM; work spread across
DVE/Act/Pool engines."""
import sys
if '/opt/trn_rl_repo' not in sys.path:
    sys.path.insert(0, '/opt/trn_rl_repo')
import json
import numpy as np
import ml_dtypes

import concourse.bass as bass
import concourse.mybir as mybir
import concourse.tile as tile
from concourse import library_config

bf16 = ml_dtypes.bfloat16
F32 = mybir.dt.float32
BF16 = mybir.dt.bfloat16
I32 = mybir.dt.int32
ALU = mybir.AluOpType
ACTF = mybir.ActivationFunctionType


def mkap(ap, dims, elem_offset=0):
    """AP with explicit [step, count] free dims (elements) after the partition dim."""
    return bass.AP(ap.tensor, ap.offset + elem_offset,
                   [list(ap.ap[0])] + [list(d) for d in dims])


def build_gat_layer(N, FIN, H, C, T_blk, n_blocks, core_rows, mode, with_bias=True,
                    tile_rows=None):
    """mode: 'elu' (layer 1) or 'mean_lsm' (layer 2)."""
    FOUT = H * C
    TCOLS = FOUT + 2 * H          # table row: [h | al_src | al_dst]
    GCOLS = FOUT + H              # gathered per edge: [h | al_src]
    NT = n_blocks * T_blk
    KCH = FIN // 128
    last_blk_rows = core_rows - (n_blocks - 1) * 128

    nc = bass.Bass("TRN2", target_bir_lowering=False, debug=False, num_devices=8)

    xT = nc.dram_tensor("xT", [128, KCH, N], BF16, kind="ExternalInput")
    wcat = nc.dram_tensor("wcat", [128, KCH, TCOLS], BF16, kind="ExternalInput")
    bcat_in = nc.dram_tensor("bcat", [128, TCOLS], BF16, kind="ExternalInput")
    iota_in = nc.dram_tensor("iota", [128, 128], BF16, kind="ExternalInput")
    ones_in = nc.dram_tensor("ones", [128, 128], BF16, kind="ExternalInput")
    srcidx_in = nc.dram_tensor("srcidx", [128, NT], I32, kind="ExternalInput")
    dstloc_in = nc.dram_tensor("dstloc", [128, NT], BF16, kind="ExternalInput")
    dstlocT_in = nc.dram_tensor("dstlocT", [NT, 128], BF16, kind="ExternalInput")
    iotac_in = nc.dram_tensor("iotac", [128, 1], BF16, kind="ExternalInput")
    adrow_in = nc.dram_tensor("adrow", [128, n_blocks], I32, kind="ExternalInput")
    if mode == "elu":
        out_d = nc.dram_tensor("out", [core_rows, FOUT], BF16, kind="ExternalOutput")
        OCOLS = FOUT
        ODT = BF16
    else:
        out_d = nc.dram_tensor("out", [core_rows, C], F32, kind="ExternalOutput")
        OCOLS = C
        ODT = F32
    table = nc.dram_tensor("table", [N, TCOLS], BF16)

    ST = 16                      # node tiles per staging buffer / table-write DMA
    CH = ST * 128                # xT chunk columns
    n_ch = (N + CH - 1) // CH

    with tile.TileContext(nc) as tc:
        with (
            tc.tile_pool(name="const", bufs=1) as kpool,
            tc.tile_pool(name="xchunk", bufs=3) as xpool,
            tc.tile_pool(name="stage", bufs=3) as stpool,
            tc.tile_pool(name="dpsum", bufs=2, space="PSUM") as dppool,
            tc.tile_pool(name="g", bufs=4) as gpool,
            tc.tile_pool(name="ad", bufs=3) as adpool,
            tc.tile_pool(name="s01", bufs=3) as spool,
            tc.tile_pool(name="ee", bufs=3) as eepool,
            tc.tile_pool(name="mp", bufs=3) as mppool,
            tc.tile_pool(name="upsum", bufs=2, space="PSUM") as uppool,
            tc.tile_pool(name="epi", bufs=2) as epool,
            tc.tile_pool(name="oacc", bufs=1) as opool,
        ):
            # ---- constants ----
            wcat_sb = kpool.tile([128, KCH * TCOLS], BF16)
            nc.sync.dma_start(out=wcat_sb[:], in_=wcat[:].rearrange("p k c -> p (k c)"))
            bcat_sb = kpool.tile([128, TCOLS], BF16)
            nc.sync.dma_start(out=bcat_sb[:], in_=bcat_in[:])
            iota_sb = kpool.tile([128, 128], BF16)
            nc.sync.dma_start(out=iota_sb[:], in_=iota_in[:])
            ones_sb = kpool.tile([128, 128], BF16)
            nc.sync.dma_start(out=ones_sb[:], in_=ones_in[:])
            srcidx_sb = kpool.tile([128, NT], I32)
            nc.sync.dma_start(out=srcidx_sb[:], in_=srcidx_in[:])
            iotac_sb = kpool.tile([128, 1], BF16)
            nc.sync.dma_start(out=iotac_sb[:], in_=iotac_in[:])
            adrow_sb = kpool.tile([128, n_blocks], I32)
            nc.sync.dma_start(out=adrow_sb[:], in_=adrow_in[:])
            dstloc_sb = kpool.tile([128, NT], BF16)
            nc.sync.dma_start(out=dstloc_sb[:], in_=dstloc_in[:])
            neg1 = kpool.tile([128, 1], F32)
            nc.vector.memset(neg1[:], -1.0)

            obuf = opool.tile([128, n_blocks * OCOLS], ODT)

            # ---- dense phase: table[N, TCOLS] = [x @ Wcat + b'] ----
            for ci in range(n_ch):
                c0 = ci * CH
                ccols = min(CH, N - c0)
                nt_ch = (ccols + 127) // 128
                xc = xpool.tile([128, KCH * CH], BF16, tag="xc")
                nc.sync.dma_start(
                    out=mkap(xc[:], [[CH, KCH], [1, ccols]]),
                    in_=xT[:, :, c0:c0 + ccols],
                )
                st = stpool.tile([128, ST * TCOLS], BF16, tag="st")
                full = ccols == CH
                if full:
                    # pairs of node tiles share one 2-bank PSUM tile; one copy per pair
                    for pr in range(ST // 2):
                        psum = dppool.tile([128, 1024], F32, tag="dp")
                        for sub in range(2):
                            tl = pr * 2 + sub
                            col = tl * 128
                            for k in range(KCH):
                                nc.tensor.matmul(
                                    psum[:, sub * 512: sub * 512 + TCOLS],
                                    lhsT=xc[:, k * CH + col: k * CH + col + 128],
                                    rhs=wcat_sb[:, k * TCOLS:(k + 1) * TCOLS],
                                    start=(k == 0),
                                    stop=(not with_bias and k == KCH - 1),
                                )
                            if with_bias:
                                nc.tensor.matmul(
                                    psum[:, sub * 512: sub * 512 + TCOLS],
                                    lhsT=ones_sb[:],
                                    rhs=bcat_sb[:],
                                    start=False, stop=True,
                                )
                        dst_ap = mkap(st[:], [[TCOLS, 2], [1, TCOLS]],
                                      elem_offset=pr * 2 * TCOLS)
                        src_ap = mkap(psum[:], [[512, 2], [1, TCOLS]])
                        eng = (ci * (ST // 2) + pr) % 2
                        if eng == 0:
                            nc.scalar.copy(out=dst_ap, in_=src_ap)
                        else:
                            nc.vector.tensor_copy(out=dst_ap, in_=src_ap)
                else:
                    for tl in range(nt_ch):
                        rows = min(128, ccols - tl * 128)
                        col = tl * 128
                        psum = dppool.tile([128, 1024], F32, tag="dp")
                        for k in range(KCH):
                            nc.tensor.matmul(
                                psum[:rows, 0:TCOLS],
                                lhsT=xc[:, k * CH + col: k * CH + col + rows],
                                rhs=wcat_sb[:, k * TCOLS:(k + 1) * TCOLS],
                                start=(k == 0),
                                stop=(not with_bias and k == KCH - 1),
                            )
                        if with_bias:
                            nc.tensor.matmul(
                                psum[:rows, 0:TCOLS],
                                lhsT=ones_sb[:, 0:rows],
                                rhs=bcat_sb[:],
                                start=False, stop=True,
                            )
                        nc.scalar.copy(
                            out=st[:rows, tl * TCOLS:(tl + 1) * TCOLS],
                            in_=psum[:rows, 0:TCOLS])
                # flush: full 128-row tiles in one strided DMA, partial tail separately
                n_full = ccols // 128
                if n_full:
                    nc.sync.dma_start(
                        out=table[c0:c0 + n_full * 128, :]
                        .rearrange("(a p) c -> p a c", p=128),
                        in_=mkap(st[:], [[TCOLS, n_full], [1, TCOLS]]),
                    )
                rem = ccols - n_full * 128
                if rem:
                    nc.sync.dma_start(
                        out=table[c0 + n_full * 128: c0 + ccols, :],
                        in_=st[:rem, n_full * TCOLS:(n_full + 1) * TCOLS],
                    )

            # ---- edge phase (software-pipelined epilogue: epi(b) after front(b+1)) ----
            table_flat = bass.AP(table[:].tensor, 0,
                                 [[N * TCOLS, 1], [1, N * TCOLS]])
            up_tiles = {}

            def edge_front(b):
                bT = b * T_blk
                # per-edge payload gathers: one indirect DMA per 128-edge tile
                g_blk = gpool.tile([128, T_blk * GCOLS], BF16, tag="g")
                for t in range(T_blk):
                    if tile_rows is None:
                        src_ap = table_flat
                    else:
                        L = int(tile_rows[b][t]) * TCOLS
                        src_ap = bass.AP(table[:].tensor, 0, [[L, 1], [1, L]])
                    nc.gpsimd.indirect_dma_start(
                        out=g_blk[:, t * GCOLS:(t + 1) * GCOLS],
                        out_offset=None,
                        in_=src_ap,
                        in_offset=bass.IndirectOffsetOnAxis(
                            ap=srcidx_sb[:, bT + t:bT + t + 1], axis=1),
                    )
                # s01[j, (t,d)] = (dstloc[j, bT+t] == d)
                s01 = spool.tile([128, T_blk * 128], BF16, tag="s01")
                nc.vector.tensor_tensor(
                    out=s01[:],
                    in0=mkap(dstloc_sb[:], [[1, T_blk], [0, 128]], elem_offset=bT),
                    in1=mkap(iota_sb[:], [[0, T_blk], [1, 128]]),
                    op=ALU.is_equal,
                )
                # al_dst broadcast to edges: S01T[d,(t,j)] from partition-bcast
                # dstlocT, then T small matmuls vs this block's al_dst rows
                rep = spool.tile([128, T_blk * 128], BF16, tag="rep")
                nc.sync.dma_start(
                    out=rep[:],
                    in_=bass.AP(dstlocT_in[:].tensor, bT * 128,
                                [[0, 128], [1, T_blk * 128]]),
                )
                s01T = spool.tile([128, T_blk * 128], BF16, tag="s01T")
                nc.vector.tensor_tensor(
                    out=s01T[:],
                    in0=rep[:],
                    in1=mkap(iotac_sb[:], [[0, T_blk], [0, 128]]),
                    op=ALU.is_equal,
                )
                adrow = adpool.tile([128, TCOLS], BF16, tag="adrow")
                nc.gpsimd.indirect_dma_start(
                    out=adrow[:], out_offset=None, in_=table_flat,
                    in_offset=bass.IndirectOffsetOnAxis(ap=adrow_sb[:, b:b + 1],
                                                        axis=1),
                )
                adp = uppool.tile([128, T_blk * H], F32, tag="adp")
                for t in range(T_blk):
                    nc.tensor.matmul(
                        adp[:, t * H:(t + 1) * H],
                        lhsT=s01T[:, t * 128:(t + 1) * 128],
                        rhs=adrow[:, FOUT + H:FOUT + 2 * H],
                        start=True, stop=True,
                    )
                # self-loop: e = lrelu(als+ald) of own row; initializes PSUM
                ssf = eepool.tile([128, H], F32, tag="ssf")
                nc.vector.tensor_tensor(
                    out=ssf[:], in0=adrow[:, FOUT:FOUT + H],
                    in1=adrow[:, FOUT + H:FOUT + 2 * H], op=ALU.add)
                slr = eepool.tile([128, H], F32, tag="slr")
                nc.vector.scalar_tensor_tensor(
                    out=slr[:], in0=ssf[:], scalar=0.2, in1=ssf[:],
                    op0=ALU.mult, op1=ALU.max)
                see = eepool.tile([128, H], F32, tag="see")
                nc.scalar.activation(see[:], slr[:], ACTF.Exp)
                # s = as + ad ; lrelu ; ee = exp
                s_f = eepool.tile([128, T_blk * H], F32, tag="sf")
                nc.vector.tensor_tensor(
                    out=s_f[:],
                    in0=mkap(g_blk[:], [[GCOLS, T_blk], [1, H]], elem_offset=FOUT),
                    in1=adp[:],
                    op=ALU.add,
                )
                lr = eepool.tile([128, T_blk * H], F32, tag="lr")
                nc.vector.scalar_tensor_tensor(
                    out=lr[:], in0=s_f[:], scalar=0.2, in1=s_f[:],
                    op0=ALU.mult, op1=ALU.max)
                ee = eepool.tile([128, T_blk * H], BF16, tag="ee")
                nc.scalar.activation(ee[:], lr[:], ACTF.Exp)
                # m' = ee*h (bf16)
                mp = mppool.tile([128, T_blk * FOUT], BF16, tag="mp")
                nc.vector.tensor_tensor(
                    out=mp[:],
                    in0=mkap(g_blk[:], [[GCOLS, T_blk], [1, FOUT]]),
                    in1=mkap(ee[:], [[H, T_blk], [1, H], [0, C]]),
                    op=ALU.mult,
                )
                # segment-sum matmuls into PSUM, initialized with the
                # self-loop contribution (ee_self * h_self | ee_self)
                up = uppool.tile([128, FOUT + H], F32, tag="up")
                up_tiles[b] = up
                nc.vector.tensor_tensor(
                    out=up[:, 0:FOUT],
                    in0=adrow[:, 0:FOUT],
                    in1=mkap(see[:], [[1, H], [0, C]]),
                    op=ALU.mult)
                nc.vector.tensor_copy(out=up[:, FOUT:FOUT + H], in_=see[:])
                for t in range(T_blk):
                    lhs = s01[:, t * 128:(t + 1) * 128]
                    nc.tensor.matmul(
                        up[:, 0:FOUT],
                        lhsT=lhs,
                        rhs=mp[:, t * FOUT:(t + 1) * FOUT],
                        start=False,
                        stop=(t == T_blk - 1),
                    )
                    nc.tensor.matmul(
                        up[:, FOUT:FOUT + H],
                        lhsT=lhs,
                        rhs=ee[:, t * H:(t + 1) * H],
                        start=False,
                        stop=(t == T_blk - 1),
                    )
            def edge_epi(b):
                brows = 128 if b < n_blocks - 1 else last_blk_rows
                up = up_tiles.pop(b)
                rec = epool.tile([128, H], F32, tag="rec")
                nc.vector.reciprocal(out=rec[:brows], in_=up[:brows, FOUT:FOUT + H])
                u = epool.tile([128, FOUT], F32, tag="u")
                for h in range(H):
                    nc.scalar.mul(u[:brows, h * C:(h + 1) * C],
                                  up[:brows, h * C:(h + 1) * C],
                                  rec[:brows, h:h + 1])
                if mode == "elu":
                    nr = epool.tile([128, FOUT], F32, tag="nr")
                    nc.scalar.activation(nr[:brows], u[:brows], ACTF.Relu, scale=-1.0)
                    ex = epool.tile([128, FOUT], F32, tag="ex")
                    nc.scalar.activation(ex[:brows], nr[:brows], ACTF.Exp, scale=-1.0)
                    sm = epool.tile([128, FOUT], F32, tag="sm")
                    nc.vector.scalar_tensor_tensor(
                        out=sm[:brows], in0=u[:brows], scalar=0.0, in1=ex[:brows],
                        op0=ALU.max, op1=ALU.add)
                    nc.scalar.activation(
                        obuf[:brows, b * OCOLS:(b + 1) * OCOLS], sm[:brows],
                        ACTF.Identity, bias=neg1[:brows])
                else:
                    m1 = epool.tile([128, FOUT // 2], F32, tag="m1")
                    nc.vector.tensor_tensor(out=m1[:brows], in0=u[:brows, 0:FOUT // 2],
                                            in1=u[:brows, FOUT // 2:FOUT], op=ALU.add)
                    m2 = epool.tile([128, FOUT // 4], F32, tag="m2")
                    nc.vector.tensor_tensor(out=m2[:brows], in0=m1[:brows, 0:FOUT // 4],
                                            in1=m1[:brows, FOUT // 4:FOUT // 2],
                                            op=ALU.add)
                    zb = epool.tile([128, C], F32, tag="zb")
                    nc.vector.tensor_tensor(out=zb[:brows], in0=m2[:brows, 0:C],
                                            in1=m2[:brows, C:2 * C], op=ALU.add)
                    zbb = epool.tile([128, C], F32, tag="zbb")
                    nc.vector.tensor_scalar_mul(out=zbb[:brows], in0=zb[:brows],
                                                scalar1=1.0 / H)
                    mxr = epool.tile([128, 1], F32, tag="mxr")
                    nc.vector.reduce_max(out=mxr[:brows], in_=zbb[:brows],
                                         axis=mybir.AxisListType.X)
                    nmx = epool.tile([128, 1], F32, tag="nmx")
                    nc.vector.tensor_scalar_mul(out=nmx[:brows], in0=mxr[:brows],
                                                scalar1=-1.0)
                    exs = epool.tile([128, C], F32, tag="exs")
                    sms = epool.tile([128, 1], F32, tag="sms")
                    nc.scalar.activation(exs[:brows], zbb[:brows], ACTF.Exp,
                                         bias=nmx[:brows], accum_out=sms[:brows])
                    lg = epool.tile([128, 1], F32, tag="lg")
                    nc.scalar.activation(lg[:brows], sms[:brows], ACTF.Ln)
                    nb = epool.tile([128, 1], F32, tag="nb")
                    nc.vector.tensor_tensor(out=nb[:brows], in0=nmx[:brows],
                                            in1=lg[:brows], op=ALU.subtract)
                    nc.scalar.activation(
                        obuf[:brows, b * OCOLS:(b + 1) * OCOLS], zbb[:brows],
                        ACTF.Identity, bias=nb[:brows])

            for b in range(n_blocks):
                edge_front(b)
                if b >= 1:
                    edge_epi(b - 1)
            edge_epi(n_blocks - 1)

            # ---- output flush: full blocks in one DMA, tail separately ----
            nfb = n_blocks - 1
            nc.sync.dma_start(
                out=out_d[0:nfb * 128, :].rearrange("(a p) c -> p a c", p=128),
                in_=mkap(obuf[:], [[OCOLS, nfb], [1, OCOLS]]),
            )
            nc.sync.dma_start(
                out=out_d[nfb * 128:core_rows, :],
                in_=obuf[:last_blk_rows, nfb * OCOLS:(nfb + 1) * OCOLS],
            )
    return nc


# ---------------- host side ----------------

def fold_weights(W, a_src, a_dst, H, C):
    """Wcat [FIN, H*C + 2H] f32: [W.T | Wa_src | Wa_dst]."""
    WT = np.asarray(W, np.float32).T.copy()           # [FIN, H*C]
    FIN = WT.shape[0]
    W3 = WT.reshape(FIN, H, C)
    Wa_s = np.einsum('fhc,hc->fh', W3, np.asarray(a_src, np.float32))
    Wa_d = np.einsum('fhc,hc->fh', W3, np.asarray(a_dst, np.float32))
    return np.concatenate([WT, Wa_s, Wa_d], axis=1)


def pack_kdim(M):
    """[FIN, COLS] -> [128, KCH, COLS]: row k*128+p -> [p, k]."""
    FIN, COLS = M.shape
    KCH = FIN // 128
    return np.ascontiguousarray(M.reshape(KCH, 128, COLS).transpose(1, 0, 2))


def route_edges(src, dst, n_cores, core_rows, n_nodes):
    """Balanced dst-node placement + per-core edge routing.

    Returns (T_blk, n_blocks, perm_pos[n_nodes], per-core dict of
    srcidx/dstidx [128,NT] i32 and dstloc [128,NT] bf16)."""
    import heapq
    n_blocks = (core_rows + 127) // 128
    NBLK = n_cores * n_blocks
    last_blk_rows = core_rows - (n_blocks - 1) * 128
    cap0 = np.full(NBLK, 128, np.int64)
    cap0[n_blocks - 1::n_blocks] = last_blk_rows

    deg = np.bincount(dst, minlength=n_nodes).astype(np.int64)
    order = np.argsort(-deg, kind='stable')
    nonself = src != dst
    src = src[nonself]
    dst = dst[nonself]

    assign_blk = np.empty(n_nodes, np.int64)
    slot_of = np.empty(n_nodes, np.int64)
    cap = cap0.copy()
    fill = np.zeros(NBLK, np.int64)
    heap = [(0, b) for b in range(NBLK)]
    heapq.heapify(heap)
    for n in order:
        s, b = heapq.heappop(heap)
        assign_blk[n] = b
        slot_of[n] = fill[b]
        fill[b] += 1
        cap[b] -= 1
        if cap[b]:
            heapq.heappush(heap, (s + deg[n], b))

    core_of_blk = assign_blk // n_blocks
    lblk_of = assign_blk % n_blocks
    perm_pos = core_of_blk * core_rows + lblk_of * 128 + slot_of

    eblk = assign_blk[dst]
    cnt = np.bincount(eblk, minlength=NBLK)
    T_blk = int(np.ceil(cnt.max() / 128.0))
    capE = T_blk * 128

    # slot edges into [NBLK, T_blk*128] padded arrays, sorted by src within
    # each block so early tiles only need early table rows
    order_e = np.lexsort((src, eblk))
    se, de = src[order_e], dst[order_e]
    blk_sorted = eblk[order_e]
    starts = np.zeros(NBLK + 1, np.int64)
    np.cumsum(cnt, out=starts[1:])
    sidx = np.zeros((NBLK, capE), np.int32)
    didx = np.zeros((NBLK, capE), np.int32)
    dloc = np.full((NBLK, capE), -1.0, np.float32)
    pos_in_blk = np.arange(len(se)) - starts[blk_sorted]
    sidx[blk_sorted, pos_in_blk] = se
    didx[blk_sorted, pos_in_blk] = de
    dloc[blk_sorted, pos_in_blk] = slot_of[de]

    # per-(block, tile) max src row, chunk-quantized, maxed across cores (SPMD)
    CHROWS = 2048
    s3 = sidx.reshape(NBLK, T_blk, 128)
    tmax = s3.max(axis=2)                                   # [NBLK, T_blk]
    tmax = np.maximum.accumulate(tmax, axis=1)
    tmax = tmax.reshape(n_cores, n_blocks, T_blk).max(axis=0)   # [n_blocks, T_blk]
    tile_rows = np.minimum((tmax // CHROWS + 1) * CHROWS, n_nodes).astype(np.int64)

    out = []
    for c in range(n_cores):
        lo, hi = c * n_blocks, (c + 1) * n_blocks
        # [n_blocks, T_blk, 128] -> [128, n_blocks*T_blk]
        s_c = sidx[lo:hi].reshape(n_blocks * T_blk, 128).T
        l2 = dloc[lo:hi].reshape(n_blocks * T_blk, 128)
        out.append({"srcidx": np.ascontiguousarray(s_c),
                    "dstloc": np.ascontiguousarray(l2.T.astype(bf16)),
                    "dstlocT": np.ascontiguousarray(l2.astype(bf16))})
    return T_blk, n_blocks, perm_pos, tile_rows, out


def scale_routed(routed, tcols, fout, hh, n_cores, core_rows, n_blocks):
    """Pre-multiply gather offsets by the table row pitch (flat-table gathers);
    build per-core adrow offsets pointing at each block's al_dst columns."""
    last = core_rows - (n_blocks - 1) * 128
    out = []
    for c, r in enumerate(routed):
        adrow = np.zeros((128, n_blocks), np.int32)
        for b in range(n_blocks):
            brows = 128 if b < n_blocks - 1 else last
            pp = np.minimum(np.arange(128), brows - 1)
            adrow[:, b] = (c * core_rows + b * 128 + pp) * tcols
        out.append({"srcidx": r["srcidx"] * np.int32(tcols),
                    "dstloc": r["dstloc"], "dstlocT": r["dstlocT"],
                    "adrow": adrow})
    return out


MAX_WAITS = 1


def fix_excess_waits(nc):
    """Post-process BIR JSON: any instruction with >MAX_WAITS sem-waits gets
    preceding Nop instructions carrying the excess waits (same engine, in-order).
    Monkeypatches nc.to_json_bytes to return the fixed JSON."""
    raw = nc.to_json_bytes()
    d = json.loads(raw)
    n_fix = 0
    for f in d["functions"]:
        for bb in f["blocks"]:
            out = []
            for inst in bb["instructions"]:
                si = inst.get("sync_info")
                waits = (si or {}).get("on_wait") or []
                if len(waits) > MAX_WAITS:
                    extra = waits[:-MAX_WAITS]
                    keep = waits[-MAX_WAITS:]
                    for ci in range(0, len(extra), MAX_WAITS):
                        chunk = extra[ci:ci + MAX_WAITS]
                        n_fix += 1
                        out.append({
                            "debug": inst.get("debug", 0),
                            "engine": inst["engine"],
                            "ins": [],
                            "is_reset_sema": False,
                            "name": f"{inst['name']}-wfix{ci}",
                            "opcode": "EventSemaphore",
                            "outs": [],
                            "sync_info": {"on_update": [], "on_wait": chunk},
                        })
                    si["on_wait"] = keep
                out.append(inst)
            bb["instructions"] = out
    fixed = json.dumps(d).encode()
    nc.to_json_bytes = lambda: fixed
    return n_fix


# ---------------- top-level kernel ----------------

N_NODES = 50000
N_CORES = 8
CORE_ROWS = N_NODES // N_CORES
_CACHE = {}


def _get_program(key, builder):
    if key not in _CACHE:
        nc = builder()
        fix_excess_waits(nc)
        _CACHE[key] = nc
    return _CACHE[key]


def _make_bcat(b, H, C, TCOLS, mode):
    """Per-row bias/128 for the ones-matmul: [b' | 0 | 0] tiled to 128 rows."""
    b = np.asarray(b, np.float32)
    row = np.zeros(TCOLS, np.float32)
    if mode == "elu":
        row[:H * C] = b
    else:
        row[:H * C] = np.tile(b, H)
    return np.tile((row / 128.0)[None, :], (128, 1)).astype(bf16)


def kernel(x, edge_index, W1, a_src1, a_dst1, b1, W2, a_src2, a_dst2, b2):
    from concourse.bass_utils import run_bass_kernel_spmd

    x = np.asarray(x, np.float32)
    ei = np.asarray(edge_index)
    N = N_NODES
    src = np.concatenate([ei[0], np.arange(N)]).astype(np.int64)
    dst = np.concatenate([ei[1], np.arange(N)]).astype(np.int64)
    T_blk, n_blocks, perm_pos, tile_rows, routed = route_edges(
        src, dst, N_CORES, CORE_ROWS, N)
    tr_key = hash(tile_rows.tobytes())

    iota_rows = np.tile(np.arange(128, dtype=np.float32)[None, :], (128, 1)).astype(bf16)
    iota_col = np.arange(128, dtype=np.float32)[:, None].astype(bf16)
    ones_sq = np.ones((128, 128), np.float32).astype(bf16)

    # ---- layer 1 ----
    H1, C1 = 8, 32
    Wcat1 = fold_weights(W1, a_src1, a_dst1, H1, C1)
    wb1 = bool(np.any(np.asarray(b1, np.float32) != 0))
    nc1 = _get_program(("l1", T_blk, wb1, tr_key), lambda: build_gat_layer(
        N, 128, H1, C1, T_blk, n_blocks, CORE_ROWS, "elu", with_bias=wb1,
        tile_rows=tile_rows))
    com1 = {
        "xT": pack_kdim(np.ascontiguousarray(x.T)).astype(bf16),
        "wcat": pack_kdim(Wcat1).astype(bf16),
        "bcat": _make_bcat(b1, H1, C1, Wcat1.shape[1], "elu"),
        "iota": iota_rows, "iotac": iota_col, "ones": ones_sq,
    }
    routed1 = scale_routed(routed, Wcat1.shape[1], H1 * C1, H1,
                           N_CORES, CORE_ROWS, n_blocks)
    in_maps1 = [dict(com1, **routed1[c]) for c in range(N_CORES)]
    res1 = run_bass_kernel_spmd(nc1, in_maps1, list(range(N_CORES)))
    h1p = np.concatenate([np.asarray(res1.results[c]["out"]) for c in range(N_CORES)],
                         axis=0)
    h1 = np.asarray(h1p, np.float32)[perm_pos]          # unpermute to node order

    # ---- layer 2 ----
    H2, C2 = 8, 40
    Wcat2 = fold_weights(W2, a_src2, a_dst2, H2, C2)
    wb2 = bool(np.any(np.asarray(b2, np.float32) != 0))
    nc2 = _get_program(("l2", T_blk, wb2, tr_key), lambda: build_gat_layer(
        N, 256, H2, C2, T_blk, n_blocks, CORE_ROWS, "mean_lsm", with_bias=wb2,
        tile_rows=tile_rows))
    com2 = {
        "xT": pack_kdim(np.ascontiguousarray(h1.T)).astype(bf16),
        "wcat": pack_kdim(Wcat2).astype(bf16),
        "bcat": _make_bcat(b2, H2, C2, Wcat2.shape[1], "mean_lsm"),
        "iota": iota_rows, "iotac": iota_col, "ones": ones_sq,
    }
    routed2 = scale_routed(routed, Wcat2.shape[1], H2 * C2, H2,
                           N_CORES, CORE_ROWS, n_blocks)
    in_maps2 = [dict(com2, **routed2[c]) for c in range(N_CORES)]
    res2 = run_bass_kernel_spmd(nc2, in_maps2, list(range(N_CORES)))
    outp = np.concatenate([np.asarray(res2.results[c]["out"]) for c in range(N_CORES)],
                          axis=0)
    return np.asarray(outp, np.float32)[perm_pos]


# revision 45
# speedup vs baseline: 1.1882x; 1.0122x over previous
"""GAT (2-layer, PyG-style) forward on 8 TRN2 NeuronCores.

Sharding: dst-node blocks across cores (host-permuted for per-block edge-count
balance); per-core edge lists routed by dst block on host; per-edge payload
gathered from a replicated node table via one batched indirect DMA per block;
al_dst broadcast via a second tiny indirect gather on the dst indices; segment
softmax + weighted sum via selection-matrix matmuls in PSUM; work spread across
DVE/Act/Pool engines."""
import sys
if '/opt/trn_rl_repo' not in sys.path:
    sys.path.insert(0, '/opt/trn_rl_repo')
import json
import numpy as np
import ml_dtypes

import concourse.bass as bass
import concourse.mybir as mybir
import concourse.tile as tile
from concourse import library_config

bf16 = ml_dtypes.bfloat16
F32 = mybir.dt.float32
BF16 = mybir.dt.bfloat16
I32 = mybir.dt.int32
ALU = mybir.AluOpType
ACTF = mybir.ActivationFunctionType


def mkap(ap, dims, elem_offset=0):
    """AP with explicit [step, count] free dims (elements) after the partition dim."""
    return bass.AP(ap.tensor, ap.offset + elem_offset,
                   [list(ap.ap[0])] + [list(d) for d in dims])


def build_gat_layer(N, FIN, H, C, T_blk, n_blocks, core_rows, mode, with_bias=True,
                    tile_rows=None):
    """mode: 'elu' (layer 1) or 'mean_lsm' (layer 2)."""
    FOUT = H * C
    TCOLS = FOUT + 2 * H          # table row: [h | al_src | al_dst]
    GCOLS = FOUT + H              # gathered per edge: [h | al_src]
    NT = n_blocks * T_blk
    KCH = FIN // 128
    last_blk_rows = core_rows - (n_blocks - 1) * 128

    nc = bass.Bass("TRN2", target_bir_lowering=False, debug=False, num_devices=8)

    xT = nc.dram_tensor("xT", [128, KCH, N], BF16, kind="ExternalInput")
    wcat = nc.dram_tensor("wcat", [128, KCH, TCOLS], BF16, kind="ExternalInput")
    bcat_in = nc.dram_tensor("bcat", [128, TCOLS], BF16, kind="ExternalInput")
    iota_in = nc.dram_tensor("iota", [128, 128], BF16, kind="ExternalInput")
    ones_in = nc.dram_tensor("ones", [128, 128], BF16, kind="ExternalInput")
    srcidx_in = nc.dram_tensor("srcidx", [128, NT], I32, kind="ExternalInput")
    dstloc_in = nc.dram_tensor("dstloc", [128, NT], BF16, kind="ExternalInput")
    dstlocT_in = nc.dram_tensor("dstlocT", [NT, 128], BF16, kind="ExternalInput")
    iotac_in = nc.dram_tensor("iotac", [128, 1], BF16, kind="ExternalInput")
    adrow_in = nc.dram_tensor("adrow", [128, n_blocks], I32, kind="ExternalInput")
    if mode == "elu":
        out_d = nc.dram_tensor("out", [core_rows, FOUT], BF16, kind="ExternalOutput")
        OCOLS = FOUT
        ODT = BF16
    else:
        out_d = nc.dram_tensor("out", [core_rows, C], F32, kind="ExternalOutput")
        OCOLS = C
        ODT = F32
    table = nc.dram_tensor("table", [N, TCOLS], BF16)

    ST = 16                      # node tiles per staging buffer / table-write DMA
    CH = ST * 128                # xT chunk columns
    n_ch = (N + CH - 1) // CH

    with tile.TileContext(nc) as tc:
        with (
            tc.tile_pool(name="const", bufs=1) as kpool,
            tc.tile_pool(name="xchunk", bufs=3) as xpool,
            tc.tile_pool(name="stage", bufs=3) as stpool,
            tc.tile_pool(name="dpsum", bufs=2, space="PSUM") as dppool,
            tc.tile_pool(name="g", bufs=5) as gpool,
            tc.tile_pool(name="ad", bufs=3) as adpool,
            tc.tile_pool(name="s01", bufs=3) as spool,
            tc.tile_pool(name="ee", bufs=3) as eepool,
            tc.tile_pool(name="mp", bufs=3) as mppool,
            tc.tile_pool(name="upsum", bufs=2, space="PSUM") as uppool,
            tc.tile_pool(name="epi", bufs=3) as epool,
            tc.tile_pool(name="oacc", bufs=1) as opool,
        ):
            # ---- constants ----
            wcat_sb = kpool.tile([128, KCH * TCOLS], BF16)
            nc.sync.dma_start(out=wcat_sb[:], in_=wcat[:].rearrange("p k c -> p (k c)"))
            bcat_sb = kpool.tile([128, TCOLS], BF16)
            nc.sync.dma_start(out=bcat_sb[:], in_=bcat_in[:])
            iota_sb = kpool.tile([128, 128], BF16)
            nc.sync.dma_start(out=iota_sb[:], in_=iota_in[:])
            ones_sb = kpool.tile([128, 128], BF16)
            nc.sync.dma_start(out=ones_sb[:], in_=ones_in[:])
            srcidx_sb = kpool.tile([128, NT], I32)
            nc.sync.dma_start(out=srcidx_sb[:], in_=srcidx_in[:])
            iotac_sb = kpool.tile([128, 1], BF16)
            nc.sync.dma_start(out=iotac_sb[:], in_=iotac_in[:])
            adrow_sb = kpool.tile([128, n_blocks], I32)
            nc.sync.dma_start(out=adrow_sb[:], in_=adrow_in[:])
            dstloc_sb = kpool.tile([128, NT], BF16)
            nc.sync.dma_start(out=dstloc_sb[:], in_=dstloc_in[:])
            neg1 = kpool.tile([128, 1], F32)
            nc.vector.memset(neg1[:], -1.0)

            obuf = opool.tile([128, n_blocks * OCOLS], ODT)

            # ---- dense phase: table[N, TCOLS] = [x @ Wcat + b'] ----
            # first chunks are small so the edge-phase gathers unlock early
            chunks = []
            c0 = 0
            for w in [512, 512, 1024]:
                chunks.append((c0, w)); c0 += w
            while c0 < N:
                w = min(CH, N - c0)
                chunks.append((c0, w)); c0 += w
            for ci, (c0, ccols) in enumerate(chunks):
                nt_ch = (ccols + 127) // 128
                xc = xpool.tile([128, KCH * CH], BF16, tag="xc")
                nc.sync.dma_start(
                    out=mkap(xc[:], [[CH, KCH], [1, ccols]]),
                    in_=xT[:, :, c0:c0 + ccols],
                )
                st = stpool.tile([128, ST * TCOLS], BF16, tag="st")
                full = ccols == CH
                if full:
                    # pairs of node tiles share one 2-bank PSUM tile; one copy per pair
                    for pr in range(ST // 2):
                        psum = dppool.tile([128, 1024], F32, tag="dp")
                        for sub in range(2):
                            tl = pr * 2 + sub
                            col = tl * 128
                            for k in range(KCH):
                                nc.tensor.matmul(
                                    psum[:, sub * 512: sub * 512 + TCOLS],
                                    lhsT=xc[:, k * CH + col: k * CH + col + 128],
                                    rhs=wcat_sb[:, k * TCOLS:(k + 1) * TCOLS],
                                    start=(k == 0),
                                    stop=(not with_bias and k == KCH - 1),
                                )
                            if with_bias:
                                nc.tensor.matmul(
                                    psum[:, sub * 512: sub * 512 + TCOLS],
                                    lhsT=ones_sb[:],
                                    rhs=bcat_sb[:],
                                    start=False, stop=True,
                                )
                        dst_ap = mkap(st[:], [[TCOLS, 2], [1, TCOLS]],
                                      elem_offset=pr * 2 * TCOLS)
                        src_ap = mkap(psum[:], [[512, 2], [1, TCOLS]])
                        eng = (ci * (ST // 2) + pr) % 2
                        if eng == 0:
                            nc.scalar.copy(out=dst_ap, in_=src_ap)
                        else:
                            nc.vector.tensor_copy(out=dst_ap, in_=src_ap)
                else:
                    for tl in range(nt_ch):
                        rows = min(128, ccols - tl * 128)
                        col = tl * 128
                        psum = dppool.tile([128, 1024], F32, tag="dp")
                        for k in range(KCH):
                            nc.tensor.matmul(
                                psum[:rows, 0:TCOLS],
                                lhsT=xc[:, k * CH + col: k * CH + col + rows],
                                rhs=wcat_sb[:, k * TCOLS:(k + 1) * TCOLS],
                                start=(k == 0),
                                stop=(not with_bias and k == KCH - 1),
                            )
                        if with_bias:
                            nc.tensor.matmul(
                                psum[:rows, 0:TCOLS],
                                lhsT=ones_sb[:, 0:rows],
                                rhs=bcat_sb[:],
                                start=False, stop=True,
                            )
                        nc.scalar.copy(
                            out=st[:rows, tl * TCOLS:(tl + 1) * TCOLS],
                            in_=psum[:rows, 0:TCOLS])
                # flush: full 128-row tiles in one strided DMA, partial tail separately
                n_full = ccols // 128
                if n_full:
                    nc.sync.dma_start(
                        out=table[c0:c0 + n_full * 128, :]
                        .rearrange("(a p) c -> p a c", p=128),
                        in_=mkap(st[:], [[TCOLS, n_full], [1, TCOLS]]),
                    )
                rem = ccols - n_full * 128
                if rem:
                    nc.sync.dma_start(
                        out=table[c0 + n_full * 128: c0 + ccols, :],
                        in_=st[:rem, n_full * TCOLS:(n_full + 1) * TCOLS],
                    )

            # ---- edge phase (software-pipelined epilogue: epi(b) after front(b+1)) ----
            table_flat = bass.AP(table[:].tensor, 0,
                                 [[N * TCOLS, 1], [1, N * TCOLS]])
            up_tiles = {}

            def edge_front(b):
                bT = b * T_blk
                # per-edge payload gathers: one indirect DMA per 128-edge tile
                g_blk = gpool.tile([128, T_blk * GCOLS], BF16, tag="g")
                for t in range(T_blk):
                    if tile_rows is None:
                        src_ap = table_flat
                    else:
                        L = int(tile_rows[b][t]) * TCOLS
                        src_ap = bass.AP(table[:].tensor, 0, [[L, 1], [1, L]])
                    nc.gpsimd.indirect_dma_start(
                        out=g_blk[:, t * GCOLS:(t + 1) * GCOLS],
                        out_offset=None,
                        in_=src_ap,
                        in_offset=bass.IndirectOffsetOnAxis(
                            ap=srcidx_sb[:, bT + t:bT + t + 1], axis=1),
                    )
                # s01[j, (t,d)] = (dstloc[j, bT+t] == d)
                s01 = spool.tile([128, T_blk * 128], BF16, tag="s01")
                nc.vector.tensor_tensor(
                    out=s01[:],
                    in0=mkap(dstloc_sb[:], [[1, T_blk], [0, 128]], elem_offset=bT),
                    in1=mkap(iota_sb[:], [[0, T_blk], [1, 128]]),
                    op=ALU.is_equal,
                )
                # al_dst broadcast to edges: S01T[d,(t,j)] from partition-bcast
                # dstlocT, then T small matmuls vs this block's al_dst rows
                rep = spool.tile([128, T_blk * 128], BF16, tag="rep")
                nc.sync.dma_start(
                    out=rep[:],
                    in_=bass.AP(dstlocT_in[:].tensor, bT * 128,
                                [[0, 128], [1, T_blk * 128]]),
                )
                s01T = spool.tile([128, T_blk * 128], BF16, tag="s01T")
                nc.vector.tensor_tensor(
                    out=s01T[:],
                    in0=rep[:],
                    in1=mkap(iotac_sb[:], [[0, T_blk], [0, 128]]),
                    op=ALU.is_equal,
                )
                adrow = adpool.tile([128, TCOLS], BF16, tag="adrow")
                nc.gpsimd.indirect_dma_start(
                    out=adrow[:], out_offset=None, in_=table_flat,
                    in_offset=bass.IndirectOffsetOnAxis(ap=adrow_sb[:, b:b + 1],
                                                        axis=1),
                )
                adp = uppool.tile([128, T_blk * H], F32, tag="adp")
                for t in range(T_blk):
                    nc.tensor.matmul(
                        adp[:, t * H:(t + 1) * H],
                        lhsT=s01T[:, t * 128:(t + 1) * 128],
                        rhs=adrow[:, FOUT + H:FOUT + 2 * H],
                        start=True, stop=True,
                    )
                # self-loop: e = lrelu(als+ald) of own row; initializes PSUM
                ssf = eepool.tile([128, H], F32, tag="ssf")
                nc.vector.tensor_tensor(
                    out=ssf[:], in0=adrow[:, FOUT:FOUT + H],
                    in1=adrow[:, FOUT + H:FOUT + 2 * H], op=ALU.add)
                slr = eepool.tile([128, H], F32, tag="slr")
                nc.vector.scalar_tensor_tensor(
                    out=slr[:], in0=ssf[:], scalar=0.2, in1=ssf[:],
                    op0=ALU.mult, op1=ALU.max)
                see = eepool.tile([128, H], F32, tag="see")
                nc.scalar.activation(see[:], slr[:], ACTF.Exp)
                # s = as + ad ; lrelu ; ee = exp
                s_f = eepool.tile([128, T_blk * H], F32, tag="sf")
                nc.vector.tensor_tensor(
                    out=s_f[:],
                    in0=mkap(g_blk[:], [[GCOLS, T_blk], [1, H]], elem_offset=FOUT),
                    in1=adp[:],
                    op=ALU.add,
                )
                lr = eepool.tile([128, T_blk * H], F32, tag="lr")
                nc.vector.scalar_tensor_tensor(
                    out=lr[:], in0=s_f[:], scalar=0.2, in1=s_f[:],
                    op0=ALU.mult, op1=ALU.max)
                ee = eepool.tile([128, T_blk * H], BF16, tag="ee")
                nc.scalar.activation(ee[:], lr[:], ACTF.Exp)
                # m' = ee*h (bf16)
                mp = mppool.tile([128, T_blk * FOUT], BF16, tag="mp")
                nc.vector.tensor_tensor(
                    out=mp[:],
                    in0=mkap(g_blk[:], [[GCOLS, T_blk], [1, FOUT]]),
                    in1=mkap(ee[:], [[H, T_blk], [1, H], [0, C]]),
                    op=ALU.mult,
                )
                # segment-sum matmuls into PSUM, initialized with the
                # self-loop contribution (ee_self * h_self | ee_self)
                up = uppool.tile([128, FOUT + H], F32, tag="up")
                up_tiles[b] = up
                nc.vector.tensor_tensor(
                    out=up[:, 0:FOUT],
                    in0=adrow[:, 0:FOUT],
                    in1=mkap(see[:], [[1, H], [0, C]]),
                    op=ALU.mult)
                nc.vector.tensor_copy(out=up[:, FOUT:FOUT + H], in_=see[:])
                for t in range(T_blk):
                    lhs = s01[:, t * 128:(t + 1) * 128]
                    nc.tensor.matmul(
                        up[:, 0:FOUT],
                        lhsT=lhs,
                        rhs=mp[:, t * FOUT:(t + 1) * FOUT],
                        start=False,
                        stop=(t == T_blk - 1),
                    )
                    nc.tensor.matmul(
                        up[:, FOUT:FOUT + H],
                        lhsT=lhs,
                        rhs=ee[:, t * H:(t + 1) * H],
                        start=False,
                        stop=(t == T_blk - 1),
                    )
            def edge_epi(b):
                brows = 128 if b < n_blocks - 1 else last_blk_rows
                up = up_tiles.pop(b)
                rec = epool.tile([128, H], F32, tag="rec")
                nc.vector.reciprocal(out=rec[:brows], in_=up[:brows, FOUT:FOUT + H])
                u = epool.tile([128, FOUT], F32, tag="u")
                for h in range(H):
                    nc.scalar.mul(u[:brows, h * C:(h + 1) * C],
                                  up[:brows, h * C:(h + 1) * C],
                                  rec[:brows, h:h + 1])
                if mode == "elu":
                    nr = epool.tile([128, FOUT], F32, tag="nr")
                    nc.scalar.activation(nr[:brows], u[:brows], ACTF.Relu, scale=-1.0)
                    ex = epool.tile([128, FOUT], F32, tag="ex")
                    nc.scalar.activation(ex[:brows], nr[:brows], ACTF.Exp, scale=-1.0)
                    sm = epool.tile([128, FOUT], F32, tag="sm")
                    nc.vector.scalar_tensor_tensor(
                        out=sm[:brows], in0=u[:brows], scalar=0.0, in1=ex[:brows],
                        op0=ALU.max, op1=ALU.add)
                    nc.scalar.activation(
                        obuf[:brows, b * OCOLS:(b + 1) * OCOLS], sm[:brows],
                        ACTF.Identity, bias=neg1[:brows])
                else:
                    m1 = epool.tile([128, FOUT // 2], F32, tag="m1")
                    nc.vector.tensor_tensor(out=m1[:brows], in0=u[:brows, 0:FOUT // 2],
                                            in1=u[:brows, FOUT // 2:FOUT], op=ALU.add)
                    m2 = epool.tile([128, FOUT // 4], F32, tag="m2")
                    nc.vector.tensor_tensor(out=m2[:brows], in0=m1[:brows, 0:FOUT // 4],
                                            in1=m1[:brows, FOUT // 4:FOUT // 2],
                                            op=ALU.add)
                    zb = epool.tile([128, C], F32, tag="zb")
                    nc.vector.tensor_tensor(out=zb[:brows], in0=m2[:brows, 0:C],
                                            in1=m2[:brows, C:2 * C], op=ALU.add)
                    zbb = epool.tile([128, C], F32, tag="zbb")
                    nc.vector.tensor_scalar_mul(out=zbb[:brows], in0=zb[:brows],
                                                scalar1=1.0 / H)
                    mxr = epool.tile([128, 1], F32, tag="mxr")
                    nc.vector.reduce_max(out=mxr[:brows], in_=zbb[:brows],
                                         axis=mybir.AxisListType.X)
                    nmx = epool.tile([128, 1], F32, tag="nmx")
                    nc.vector.tensor_scalar_mul(out=nmx[:brows], in0=mxr[:brows],
                                                scalar1=-1.0)
                    exs = epool.tile([128, C], F32, tag="exs")
                    sms = epool.tile([128, 1], F32, tag="sms")
                    nc.scalar.activation(exs[:brows], zbb[:brows], ACTF.Exp,
                                         bias=nmx[:brows], accum_out=sms[:brows])
                    lg = epool.tile([128, 1], F32, tag="lg")
                    nc.scalar.activation(lg[:brows], sms[:brows], ACTF.Ln)
                    nb = epool.tile([128, 1], F32, tag="nb")
                    nc.vector.tensor_tensor(out=nb[:brows], in0=nmx[:brows],
                                            in1=lg[:brows], op=ALU.subtract)
                    nc.scalar.activation(
                        obuf[:brows, b * OCOLS:(b + 1) * OCOLS], zbb[:brows],
                        ACTF.Identity, bias=nb[:brows])

            for b in range(n_blocks):
                edge_front(b)
                if b >= 1:
                    edge_epi(b - 1)
            edge_epi(n_blocks - 1)

            # ---- output flush: full blocks in one DMA, tail separately ----
            nfb = n_blocks - 1
            nc.sync.dma_start(
                out=out_d[0:nfb * 128, :].rearrange("(a p) c -> p a c", p=128),
                in_=mkap(obuf[:], [[OCOLS, nfb], [1, OCOLS]]),
            )
            nc.sync.dma_start(
                out=out_d[nfb * 128:core_rows, :],
                in_=obuf[:last_blk_rows, nfb * OCOLS:(nfb + 1) * OCOLS],
            )
    return nc


# ---------------- host side ----------------

def fold_weights(W, a_src, a_dst, H, C):
    """Wcat [FIN, H*C + 2H] f32: [W.T | Wa_src | Wa_dst]."""
    WT = np.asarray(W, np.float32).T.copy()           # [FIN, H*C]
    FIN = WT.shape[0]
    W3 = WT.reshape(FIN, H, C)
    Wa_s = np.einsum('fhc,hc->fh', W3, np.asarray(a_src, np.float32))
    Wa_d = np.einsum('fhc,hc->fh', W3, np.asarray(a_dst, np.float32))
    return np.concatenate([WT, Wa_s, Wa_d], axis=1)


def pack_kdim(M):
    """[FIN, COLS] -> [128, KCH, COLS]: row k*128+p -> [p, k]."""
    FIN, COLS = M.shape
    KCH = FIN // 128
    return np.ascontiguousarray(M.reshape(KCH, 128, COLS).transpose(1, 0, 2))


def route_edges(src, dst, n_cores, core_rows, n_nodes):
    """Balanced dst-node placement + per-core edge routing.

    Returns (T_blk, n_blocks, perm_pos[n_nodes], per-core dict of
    srcidx/dstidx [128,NT] i32 and dstloc [128,NT] bf16)."""
    import heapq
    n_blocks = (core_rows + 127) // 128
    NBLK = n_cores * n_blocks
    last_blk_rows = core_rows - (n_blocks - 1) * 128
    cap0 = np.full(NBLK, 128, np.int64)
    cap0[n_blocks - 1::n_blocks] = last_blk_rows

    deg = np.bincount(dst, minlength=n_nodes).astype(np.int64)
    order = np.argsort(-deg, kind='stable')
    nonself = src != dst
    src = src[nonself]
    dst = dst[nonself]

    assign_blk = np.empty(n_nodes, np.int64)
    slot_of = np.empty(n_nodes, np.int64)
    cap = cap0.copy()
    fill = np.zeros(NBLK, np.int64)
    heap = [(0, b) for b in range(NBLK)]
    heapq.heapify(heap)
    for n in order:
        s, b = heapq.heappop(heap)
        assign_blk[n] = b
        slot_of[n] = fill[b]
        fill[b] += 1
        cap[b] -= 1
        if cap[b]:
            heapq.heappush(heap, (s + deg[n], b))

    core_of_blk = assign_blk // n_blocks
    lblk_of = assign_blk % n_blocks
    perm_pos = core_of_blk * core_rows + lblk_of * 128 + slot_of

    eblk = assign_blk[dst]
    cnt = np.bincount(eblk, minlength=NBLK)
    T_blk = int(np.ceil(cnt.max() / 128.0))
    capE = T_blk * 128

    # slot edges into [NBLK, T_blk*128] padded arrays, sorted by src within
    # each block so early tiles only need early table rows
    order_e = np.lexsort((src, eblk))
    se, de = src[order_e], dst[order_e]
    blk_sorted = eblk[order_e]
    starts = np.zeros(NBLK + 1, np.int64)
    np.cumsum(cnt, out=starts[1:])
    sidx = np.zeros((NBLK, capE), np.int32)
    didx = np.zeros((NBLK, capE), np.int32)
    dloc = np.full((NBLK, capE), -1.0, np.float32)
    pos_in_blk = np.arange(len(se)) - starts[blk_sorted]
    sidx[blk_sorted, pos_in_blk] = se
    didx[blk_sorted, pos_in_blk] = de
    dloc[blk_sorted, pos_in_blk] = slot_of[de]

    # per-(block, tile) max src row, chunk-quantized, maxed across cores (SPMD)
    CHROWS = 2048
    s3 = sidx.reshape(NBLK, T_blk, 128)
    tmax = s3.max(axis=2)                                   # [NBLK, T_blk]
    tmax = np.maximum.accumulate(tmax, axis=1)
    tmax = tmax.reshape(n_cores, n_blocks, T_blk).max(axis=0)   # [n_blocks, T_blk]
    tile_rows = np.minimum((tmax // CHROWS + 1) * CHROWS, n_nodes).astype(np.int64)

    out = []
    for c in range(n_cores):
        lo, hi = c * n_blocks, (c + 1) * n_blocks
        # [n_blocks, T_blk, 128] -> [128, n_blocks*T_blk]
        s_c = sidx[lo:hi].reshape(n_blocks * T_blk, 128).T
        l2 = dloc[lo:hi].reshape(n_blocks * T_blk, 128)
        out.append({"srcidx": np.ascontiguousarray(s_c),
                    "dstloc": np.ascontiguousarray(l2.T.astype(bf16)),
                    "dstlocT": np.ascontiguousarray(l2.astype(bf16))})
    return T_blk, n_blocks, perm_pos, tile_rows, out


def scale_routed(routed, tcols, fout, hh, n_cores, core_rows, n_blocks):
    """Pre-multiply gather offsets by the table row pitch (flat-table gathers);
    build per-core adrow offsets pointing at each block's al_dst columns."""
    last = core_rows - (n_blocks - 1) * 128
    out = []
    for c, r in enumerate(routed):
        adrow = np.zeros((128, n_blocks), np.int32)
        for b in range(n_blocks):
            brows = 128 if b < n_blocks - 1 else last
            pp = np.minimum(np.arange(128), brows - 1)
            adrow[:, b] = (c * core_rows + b * 128 + pp) * tcols
        out.append({"srcidx": r["srcidx"] * np.int32(tcols),
                    "dstloc": r["dstloc"], "dstlocT": r["dstlocT"],
                    "adrow": adrow})
    return out


MAX_WAITS = 1


def fix_excess_waits(nc):
    """Post-process BIR JSON: any instruction with >MAX_WAITS sem-waits gets
    preceding Nop instructions carrying the excess waits (same engine, in-order).
    Monkeypatches nc.to_json_bytes to return the fixed JSON."""
    raw = nc.to_json_bytes()
    d = json.loads(raw)
    n_fix = 0
    for f in d["functions"]:
        for bb in f["blocks"]:
            out = []
            for inst in bb["instructions"]:
                si = inst.get("sync_info")
                waits = (si or {}).get("on_wait") or []
                if len(waits) > MAX_WAITS:
                    extra = waits[:-MAX_WAITS]
                    keep = waits[-MAX_WAITS:]
                    for ci in range(0, len(extra), MAX_WAITS):
                        chunk = extra[ci:ci + MAX_WAITS]
                        n_fix += 1
                        out.append({
                            "debug": inst.get("debug", 0),
                            "engine": inst["engine"],
                            "ins": [],
                            "is_reset_sema": False,
                            "name": f"{inst['name']}-wfix{ci}",
                            "opcode": "EventSemaphore",
                            "outs": [],
                            "sync_info": {"on_update": [], "on_wait": chunk},
                        })
                    si["on_wait"] = keep
                out.append(inst)
            bb["instructions"] = out
    fixed = json.dumps(d).encode()
    nc.to_json_bytes = lambda: fixed
    return n_fix


# ---------------- top-level kernel ----------------

N_NODES = 50000
N_CORES = 8
CORE_ROWS = N_NODES // N_CORES
_CACHE = {}


def _get_program(key, builder):
    if key not in _CACHE:
        nc = builder()
        fix_excess_waits(nc)
        _CACHE[key] = nc
    return _CACHE[key]


def _make_bcat(b, H, C, TCOLS, mode):
    """Per-row bias/128 for the ones-matmul: [b' | 0 | 0] tiled to 128 rows."""
    b = np.asarray(b, np.float32)
    row = np.zeros(TCOLS, np.float32)
    if mode == "elu":
        row[:H * C] = b
    else:
        row[:H * C] = np.tile(b, H)
    return np.tile((row / 128.0)[None, :], (128, 1)).astype(bf16)


def kernel(x, edge_index, W1, a_src1, a_dst1, b1, W2, a_src2, a_dst2, b2):
    from concourse.bass_utils import run_bass_kernel_spmd

    x = np.asarray(x, np.float32)
    ei = np.asarray(edge_index)
    N = N_NODES
    src = np.concatenate([ei[0], np.arange(N)]).astype(np.int64)
    dst = np.concatenate([ei[1], np.arange(N)]).astype(np.int64)
    T_blk, n_blocks, perm_pos, tile_rows, routed = route_edges(
        src, dst, N_CORES, CORE_ROWS, N)
    tr_key = hash(tile_rows.tobytes())

    iota_rows = np.tile(np.arange(128, dtype=np.float32)[None, :], (128, 1)).astype(bf16)
    iota_col = np.arange(128, dtype=np.float32)[:, None].astype(bf16)
    ones_sq = np.ones((128, 128), np.float32).astype(bf16)

    # ---- layer 1 ----
    H1, C1 = 8, 32
    Wcat1 = fold_weights(W1, a_src1, a_dst1, H1, C1)
    wb1 = bool(np.any(np.asarray(b1, np.float32) != 0))
    nc1 = _get_program(("l1", T_blk, wb1, tr_key), lambda: build_gat_layer(
        N, 128, H1, C1, T_blk, n_blocks, CORE_ROWS, "elu", with_bias=wb1,
        tile_rows=tile_rows))
    com1 = {
        "xT": pack_kdim(np.ascontiguousarray(x.T)).astype(bf16),
        "wcat": pack_kdim(Wcat1).astype(bf16),
        "bcat": _make_bcat(b1, H1, C1, Wcat1.shape[1], "elu"),
        "iota": iota_rows, "iotac": iota_col, "ones": ones_sq,
    }
    routed1 = scale_routed(routed, Wcat1.shape[1], H1 * C1, H1,
                           N_CORES, CORE_ROWS, n_blocks)
    in_maps1 = [dict(com1, **routed1[c]) for c in range(N_CORES)]
    res1 = run_bass_kernel_spmd(nc1, in_maps1, list(range(N_CORES)))
    h1p = np.concatenate([np.asarray(res1.results[c]["out"]) for c in range(N_CORES)],
                         axis=0)
    h1 = np.asarray(h1p, np.float32)[perm_pos]          # unpermute to node order

    # ---- layer 2 ----
    H2, C2 = 8, 40
    Wcat2 = fold_weights(W2, a_src2, a_dst2, H2, C2)
    wb2 = bool(np.any(np.asarray(b2, np.float32) != 0))
    nc2 = _get_program(("l2", T_blk, wb2, tr_key), lambda: build_gat_layer(
        N, 256, H2, C2, T_blk, n_blocks, CORE_ROWS, "mean_lsm", with_bias=wb2,
        tile_rows=tile_rows))
    com2 = {
        "xT": pack_kdim(np.ascontiguousarray(h1.T)).astype(bf16),
        "wcat": pack_kdim(Wcat2).astype(bf16),
        "bcat": _make_bcat(b2, H2, C2, Wcat2.shape[1], "mean_lsm"),
        "iota": iota_rows, "iotac": iota_col, "ones": ones_sq,
    }
    routed2 = scale_routed(routed, Wcat2.shape[1], H2 * C2, H2,
                           N_CORES, CORE_ROWS, n_blocks)
    in_maps2 = [dict(com2, **routed2[c]) for c in range(N_CORES)]
    res2 = run_bass_kernel_spmd(nc2, in_maps2, list(range(N_CORES)))
    outp = np.concatenate([np.asarray(res2.results[c]["out"]) for c in range(N_CORES)],
                          axis=0)
    return np.asarray(outp, np.float32)[perm_pos]
